# revision 20
# baseline (speedup 1.0000x reference)
"""AttentionDecoder Trainium2 Bass kernel.

Shapes (hardcoded): B=4, L=256, H=256, HEADS=4, D=64, BLOCKS=2.

Sharding: 8 cores; core c owns (batch b = c//2, query-half qh = c%2), i.e.
128 query rows x all 4 heads. Params replicated. Time matrices are sharded
on (batch, query) and streamed through the PE as the *stationary* operand
(fast-weight-load path), producing transposed logits w^T[k,q] / outputs
o^T[hd,q] whose per-query results land in PSUM *columns* (PE outputs must
start at 32-aligned partitions, so per-query row writes are not allowed).
An AllGather over core pairs exchanges updated activations between blocks;
the tmK matmuls of block 2 only need block-2 Q and the SBUF-resident time
matrices, so they execute during the exchange.

Host-side prep (free): layout transposes, folding ln/softmax-scale into
weights, casting the time-matrix stream to fp8, packing all small consts
into two DMA-able panels.
"""
import os
import sys

import numpy as np
import ml_dtypes

for _p in ("/opt/trn_rl_repo", os.path.expanduser("~/.axon_site/_ro/trn_rl_repo")):
    if os.path.isdir(_p) and _p not in sys.path:
        sys.path.insert(0, _p)
        break

import concourse.bacc as bacc
import concourse.mybir as mybir
import concourse.tile as tile
from concourse.bass_utils import run_bass_kernel_spmd

B, L, H, HEADS, BLOCKS = 4, 256, 256, 4, 2
D = 64
NC = 8
EPS = 1e-8
NEG = -4294967295.0
SCALE = 0.125

TM_FP8 = True  # time-matrix stream dtype: fp8e4m3 / bf16

F32 = mybir.dt.float32
BF = mybir.dt.bfloat16
TMDT = mybir.dt.float8e4 if TM_FP8 else BF
NPBF = ml_dtypes.bfloat16
TM_NP = ml_dtypes.float8_e4m3fn if TM_FP8 else NPBF
AF = mybir.ActivationFunctionType
OP = mybir.AluOpType
AG_GROUPS = [[0, 1], [2, 3], [4, 5], [6, 7]]

# f32 const-panel column offsets
MNEG_C, NPAD_C, LNL_C, QB_C, B1_C, B2_C, EPS_C, ID_C = 0, 256, 257, 513, 517, 521, 525, 526
F32P_COLS = 654
# bf16 const-panel column offsets
QW_C, KW_C, VW_C, W1_C, W2_C, APK_C, APV_C, ONES_C = 0, 1024, 2048, 3072, 4096, 5120, 6144, 7168
BFP_COLS = 7169


def _f32(x):
    return np.ascontiguousarray(x, np.float32)


def _prep_core(inp, c):
    """Host-side layout prep for core c. Pure data movement + dtype casts."""
    b, qh = c // 2, c % 2
    qs = slice(qh * 128, qh * 128 + 128)
    m = {}
    m["seqs_tok"] = _f32(inp["seqs"][b, qs, :])
    m["seqsT"] = _f32(inp["seqs"][b]).T.reshape(2, 128, 256).astype(NPBF)
    # tmK chunks [128(2q x 64d), 128 k-half]: unit u=(h,oct), col (jj*2+kh)*128;
    # 4 units per 1MB macro row-contiguous DMA
    arr = _f32(inp["time_matrix_K"][b, qs])  # [128q,256k,256h]
    a4 = arr.reshape(64, 2, 256, 4, 64).transpose(3, 0, 1, 4, 2).reshape(4, 64, 128, 256)
    units = a4.reshape(4, 8, 8, 128, 2, 128).transpose(0, 1, 3, 2, 4, 5).reshape(32, 128, 2048)
    m["tmK"] = np.ascontiguousarray(
        units.reshape(4, 8, 128, 2048).transpose(0, 2, 1, 3).reshape(4, 128, 16384)
    ).astype(TM_NP)
    # tmV chunks [128 k-half, 128 hd-half]: unit u=q//4, col (qq*4+kh*2+hh)*128
    arr = _f32(inp["time_matrix_V"][b, qs])  # [128q,256k,256hd]
    u6 = arr.reshape(32, 4, 2, 128, 2, 128).transpose(0, 3, 1, 2, 4, 5).reshape(32, 128, 2048)
    m["tmV"] = np.ascontiguousarray(
        u6.reshape(4, 8, 128, 2048).transpose(0, 2, 1, 3).reshape(4, 128, 16384)
    ).astype(TM_NP)

    am = np.asarray(inp["attention_mask"], bool)
    tlm = np.asarray(inp["timeline_mask"], bool)
    mneg = _f32(np.where(tlm[b, qs][:, None] | am[qs, :], NEG, 0.0))
    mnegT = _f32(mneg.T.reshape(2, 128, 128))  # [kh, k, q]
    npad = _f32(1.0 - tlm[b, qs].astype(np.float32))[:, None]
    lnl = _f32(np.broadcast_to(inp["ln_last"], (128, 256)))

    f32_parts = [mnegT[0], mnegT[1], npad, lnl]
    qb_parts, b1_parts, b2_parts = [], [], []
    qw_parts, kw_parts, vw_parts, w1_parts, w2_parts, apk_parts, apv_parts = ([] for _ in range(7))
    for i in range(BLOCKS):
        qw_eff = (_f32(inp["Qw"][i]) * _f32(inp["ln_attn"][i])[None, :] * SCALE).T  # [hin,hcol]
        qw4 = qw_eff.reshape(2, 128, 2, 128).transpose(0, 2, 1, 3)  # [a,t,128,128]
        for a in range(2):
            for t in range(2):
                qw_parts.append(qw4[a, t])
        for t in range(2):
            qb_parts.append(_f32(inp["Qb"][i] * SCALE).reshape(2, 128, 1)[t])
            b1_parts.append(_f32(inp["b1"][i]).reshape(2, 128, 1)[t])
            b2_parts.append(_f32(inp["b2"][i]).reshape(2, 128, 1)[t])
        kw_parts.extend(_f32(inp["Kw"][i]).T.reshape(2, 128, 256))
        vw_parts.extend(_f32(inp["Vw"][i]).T.reshape(2, 128, 256))
        w1_eff = (_f32(inp["W1"][i]) * _f32(inp["ln_ffn"][i])[None, :]).T
        w1_parts.extend(w1_eff.reshape(2, 128, 256))
        w2_parts.extend(_f32(inp["W2"][i]).T.reshape(2, 128, 256))
        apk_parts.extend((_f32(inp["abs_pos_K"][b]).T + _f32(inp["Kb"][i])[:, None]).reshape(2, 128, 256))
        apv_parts.extend((_f32(inp["abs_pos_V"][b]) + _f32(inp["Vb"][i])[None, :]).reshape(2, 128, 256))
    f32_parts += qb_parts + b1_parts + b2_parts
    f32_parts.append(np.full((128, 1), EPS, np.float32))
    f32_parts.append(_f32(np.eye(128)))
    f32p = np.concatenate(f32_parts, axis=1)
    assert f32p.shape == (128, F32P_COLS), f32p.shape
    m["f32p"] = _f32(f32p)
    bf_parts = (qw_parts + kw_parts + vw_parts + w1_parts + w2_parts +
                apk_parts + apv_parts + [np.ones((128, 1), np.float32)])
    bfp = np.concatenate(bf_parts, axis=1)
    assert bfp.shape == (128, BFP_COLS), bfp.shape
    m["bfp"] = bfp.astype(NPBF)
    return m


def _build():
    nc = bacc.Bacc("TRN2", target_bir_lowering=False, debug=False, num_devices=NC)

    def dp(name, shape, dt):
        return nc.dram_tensor(name, list(shape), dt, kind="ExternalInput").ap()

    d_seqs = dp("seqs_tok", (128, 256), F32)
    d_seqsT = dp("seqsT", (2, 128, 256), BF)
    d_f32p = dp("f32p", (128, F32P_COLS), F32)
    d_bfp = dp("bfp", (128, BFP_COLS), BF)
    d_tmK = dp("tmK", (4, 128, 16384), TMDT)
    d_tmV = dp("tmV", (4, 128, 16384), TMDT)
    d_out = nc.dram_tensor("out", [128, 256], F32, kind="ExternalOutput").ap()

    with tile.TileContext(nc) as tc:
        with (
            tc.tile_pool(name="wts", bufs=1) as wp,
            tc.tile_pool(name="act", bufs=1) as acp,
            tc.tile_pool(name="rot", bufs=2) as rp,
            tc.tile_pool(name="tmres", bufs=1) as tres,
            tc.tile_pool(name="psw", bufs=2, space="PSUM") as psw,
            tc.tile_pool(name="pso", bufs=2, space="PSUM") as pso,
            tc.tile_pool(name="pst", bufs=2, space="PSUM") as pst,
            tc.tile_pool(name="psp", bufs=2, space="PSUM") as psp,
            tc.tile_pool(name="dram", bufs=1, space="DRAM") as dpool,
        ):
            # activations + const panels first — nothing queues behind the
            # 16.8MB time-matrix burst
            seqs_cur = acp.tile([128, 256], F32, name="seqs0", tag="seqs0")
            nc.sync.dma_start(out=seqs_cur[:], in_=d_seqs)
            sT = []
            for a in range(2):
                t = acp.tile([128, 256], BF, name=f"sT0_{a}", tag=f"sT0_{a}")
                nc.sync.dma_start(out=t[:], in_=d_seqsT[a])
                sT.append(t)
            f32p = wp.tile([128, F32P_COLS], F32, name="f32p", tag="f32p")
            nc.sync.dma_start(out=f32p[:], in_=d_f32p)
            bfp = wp.tile([128, BFP_COLS], BF, name="bfp", tag="bfp")
            nc.sync.dma_start(out=bfp[:], in_=d_bfp)

            mnegT = lambda kh: f32p[:, MNEG_C + 128 * kh:MNEG_C + 128 * (kh + 1)]
            npad = f32p[:, NPAD_C:NPAD_C + 1]
            lnl = f32p[:, LNL_C:LNL_C + 256]
            qb = lambda i, t: f32p[:, QB_C + 2 * i + t:QB_C + 2 * i + t + 1]
            b1 = lambda i, t: f32p[:, B1_C + 2 * i + t:B1_C + 2 * i + t + 1]
            b2 = lambda i, t: f32p[:, B2_C + 2 * i + t:B2_C + 2 * i + t + 1]
            epsb = f32p[:, EPS_C:EPS_C + 1]
            ident = f32p[:, ID_C:ID_C + 128]
            qw = lambda i, a, t: bfp[:, QW_C + ((i * 2 + a) * 2 + t) * 128:
                                     QW_C + ((i * 2 + a) * 2 + t + 1) * 128]
            kw = lambda i, a: bfp[:, KW_C + (i * 2 + a) * 256:KW_C + (i * 2 + a + 1) * 256]
            vw = lambda i, a: bfp[:, VW_C + (i * 2 + a) * 256:VW_C + (i * 2 + a + 1) * 256]
            w1 = lambda i, a: bfp[:, W1_C + (i * 2 + a) * 256:W1_C + (i * 2 + a + 1) * 256]
            w2 = lambda i, a: bfp[:, W2_C + (i * 2 + a) * 256:W2_C + (i * 2 + a + 1) * 256]
            apk = lambda i, t: bfp[:, APK_C + (i * 2 + t) * 256:APK_C + (i * 2 + t + 1) * 256]
            apv = lambda i, t: bfp[:, APV_C + (i * 2 + t) * 256:APV_C + (i * 2 + t + 1) * 256]
            ones = bfp[:, ONES_C:ONES_C + 1]

            # resident tm macros (4 units each, loaded once, both blocks use
            # them); issue stays on sync so no compute engine stalls on DMA
            # queue backpressure
            tmk_res, tmv_res = [], []
            for mi in range(4):
                t = tres.tile([128, 16384], TMDT, name=f"rk{mi}", tag=f"rk{mi}")
                nc.sync.dma_start(out=t[:], in_=d_tmK[mi])
                tmk_res.append(t)
            for mi in range(4):
                t = tres.tile([128, 16384], TMDT, name=f"rv{mi}", tag=f"rv{mi}")
                nc.gpsimd.dma_start(out=t[:], in_=d_tmV[mi])
                tmv_res.append(t)

            def tmk_chunk(u, jj, kh):
                c0 = (u % 8) * 2048 + (jj * 2 + kh) * 128
                return tmk_res[u // 8][:, c0:c0 + 128]

            def tmv_chunk(u, qq, kh, hh):
                c0 = (u % 8) * 2048 + (qq * 4 + kh * 2 + hh) * 128
                return tmv_res[u // 8][:, c0:c0 + 128]

            def rmsnorm_rs(i, src, label):
                """[128,1] f32 tile holding 1/sqrt(mean(src^2)+EPS)."""
                scr = rp.tile([128, 256], F32, name=f"scr_{label}{i}", tag="scr")
                ssum = rp.tile([128, 1], F32, name=f"ss_{label}{i}", tag="ss")
                nc.scalar.activation(scr[:], src[:], AF.Square, accum_out=ssum[:])
                st_ = rp.tile([128, 1], F32, name=f"st_{label}{i}", tag="st")
                nc.scalar.activation(st_[:], ssum[:], AF.Sqrt, scale=1.0 / 256.0,
                                     bias=epsb)
                rs_ = rp.tile([128, 1], F32, name=f"rs_{label}{i}", tag="rs")
                nc.vector.reciprocal(rs_[:], st_[:])
                return rs_

            def transpose_pair(i, src, label, out_dt):
                """[128,256] f32 -> two [128,128] out_dt transposed tiles."""
                outs = []
                for a in range(2):
                    tp = pst.tile([128, 128], F32, name=f"tp_{label}{i}{a}", tag="tr")
                    nc.tensor.transpose(tp[:], src[:, 128 * a:128 * (a + 1)], ident)
                    ot = rp.tile([128, 128], out_dt, name=f"{label}T{i}{a}", tag=f"{label}T{a}")
                    nc.vector.tensor_copy(ot[:], tp[:])
                    outs.append(ot)
                return outs

            hf = None
            for i in range(BLOCKS):
                # ---- rmsnorm (Q path) + Q projection (local data only) ----
                rs_ = rmsnorm_rs(i, seqs_cur, "q")
                x_sb = rp.tile([128, 256], F32, name=f"x{i}", tag="x")
                nc.vector.tensor_scalar_mul(x_sb[:], seqs_cur[:], rs_[:])
                xT = transpose_pair(i, x_sb, "x", BF)
                QT = []
                for t in range(2):
                    pp = psp.tile([128, 128], F32, name=f"qps{i}{t}", tag="proj")
                    for a in range(2):
                        nc.tensor.matmul(pp[:], qw(i, a, t), xT[a][:],
                                         start=(a == 0), stop=(a == 1))
                    qt = acp.tile([128, 128], BF, name=f"QT{i}{t}", tag=f"QT{i}{t}")
                    nc.scalar.activation(qt[:], pp[:], AF.Identity, bias=qb(i, t))
                    QT.append(qt)
                # block-diagonal packed Q (moving operand for the tmK matvecs)
                QS = []
                for h in range(4):
                    q = acp.tile([128, 128], TMDT, name=f"QS{i}{h}", tag=f"QS{i}{h}")
                    nc.vector.memset(q[:], 0.0)
                    src = QT[h // 2][64 * (h % 2):64 * (h % 2) + 64, :]
                    nc.vector.tensor_copy(q[0:64, 0:128:2], src[:, 0:128:2])
                    nc.vector.tensor_copy(q[64:128, 1:128:2], src[:, 1:128:2])
                    QS.append(q)

                # ---- w3: A . tmK, all heads, packed [128k, (e,kh)*128+2q] ----
                # needs only Q + resident tm: block 2's run during the
                # inter-block exchange
                wTb = [psw.tile([128, 512], F32, name=f"wb{i}{hp}", tag="w")
                       for hp in range(2)]
                for h in range(4):
                    hp, e = h // 2, h % 2
                    for kh in range(2):
                        cb = (e * 2 + kh) * 128
                        for j in range(64):
                            u, jj = h * 8 + j // 8, j % 8
                            nc.tensor.matmul(wTb[hp][:, cb + 2 * j:cb + 2 * j + 2],
                                             tmk_chunk(u, jj, kh),
                                             QS[h][:, 2 * j:2 * j + 2],
                                             start=True, stop=True,
                                             skip_group_check=True)

                # ---- K/V projections (block 2: needs exchanged seqsT) ----
                if i > 0:
                    sT = [acp.tile([128, 256], BF, name=f"sT{i}_{a}", tag=f"sT{i}_{a}")
                          for a in range(2)]
                    for a in range(2):
                        for half in range(2):
                            tp = pst.tile([128, 128], F32, name=f"tpe{i}{a}{half}", tag="tr")
                            nc.tensor.transpose(tp[:], hf[half][:, 128 * a:128 * (a + 1)],
                                                ident)
                            nc.vector.tensor_copy(sT[a][:, 128 * half:128 * (half + 1)], tp[:])
                KapT = []
                for t in range(2):
                    pp = psp.tile([128, 256], F32, name=f"kps{i}{t}", tag="proj")
                    for a in range(2):
                        nc.tensor.matmul(pp[:], kw(i, a)[:, 128 * t:128 * (t + 1)],
                                         sT[a][:], start=(a == 0), stop=(a == 1))
                    kt = acp.tile([128, 256], BF, name=f"KapT{i}{t}", tag=f"KapT{i}{t}")
                    nc.vector.tensor_add(kt[:], pp[:], apk(i, t))
                    KapT.append(kt)
                VapV = []
                for t in range(2):
                    pp = psp.tile([128, 256], F32, name=f"vps{i}{t}", tag="proj")
                    for a in range(2):
                        nc.tensor.matmul(pp[:], sT[a][:, 128 * t:128 * (t + 1)],
                                         vw(i, a), start=(a == 0), stop=(a == 1))
                    vt = acp.tile([128, 256], BF, name=f"VapV{i}{t}", tag=f"VapV{i}{t}")
                    nc.vector.tensor_add(vt[:], pp[:], apv(i, t))
                    VapV.append(vt)

                # ---- w12 + mask + softmax (transposed layout) ----
                ATp = [[acp.tile([128, 256], TMDT, name=f"ATp{i}{hp}{kh}", tag=f"ATp{i}{hp}{kh}")
                        for kh in range(2)] for hp in range(2)]
                ATn = [[acp.tile([128, 128], BF, name=f"ATn{i}{h}{kh}", tag=f"ATn{i}{h}{kh}")
                        for kh in range(2)] for h in range(4)]
                sps = pst.tile([128, 4], F32, name=f"sps{i}", tag="tr")
                for h in range(4):
                    hp, e = h // 2, h % 2
                    for kh in range(2):
                        w12 = psp.tile([128, 128], F32, name=f"w12_{i}{h}{kh}", tag="proj")
                        lhs = KapT[h // 2][64 * (h % 2):64 * (h % 2) + 64,
                                           128 * kh:128 * (kh + 1)]
                        rhs = QT[h // 2][64 * (h % 2):64 * (h % 2) + 64, :]
                        nc.tensor.matmul(w12[:], lhs, rhs, start=True, stop=True)
                        wmk = rp.tile([128, 128], F32, name=f"wmk{i}{h}{kh}", tag="wmk")
                        nc.vector.tensor_add(wmk[:], w12[:], mnegT(kh))
                        wm = rp.tile([128, 128], F32, name=f"wm{i}{h}{kh}", tag="wm")
                        nc.vector.tensor_add(
                            wm[:], wTb[hp][:, (e * 2 + kh) * 128:(e * 2 + kh + 1) * 128],
                            wmk[:])
                        nc.scalar.activation(ATp[hp][kh][:, e:256:2], wm[:], AF.Exp)
                        nc.scalar.activation(ATn[h][kh][:], wm[:], AF.Exp)
                        # per-query row sums: sps[:, h] += ATn^T @ ones
                        nc.tensor.matmul(sps[:, h:h + 1], ATn[h][kh][:], ones,
                                         start=(kh == 0), stop=(kh == 1),
                                         skip_group_check=True)
                rr = []
                for h in range(4):
                    r_h = acp.tile([128, 1], F32, name=f"r{i}{h}", tag=f"r{i}{h}")
                    nc.vector.reciprocal(r_h[:], sps[:, h:h + 1])
                    rr.append(r_h)

                # ---- attention output (transposed accumulation) ----
                oT = []
                for hp in range(2):
                    pp = psp.tile([128, 128], F32, name=f"o12t{i}{hp}", tag="proj")
                    for e in range(2):
                        h = 2 * hp + e
                        for kh in range(2):
                            nc.tensor.matmul(pp[64 * e:64 * (e + 1), :],
                                             VapV[kh][:, 64 * h:64 * (h + 1)],
                                             ATn[h][kh][:],
                                             start=(kh == 0), stop=(kh == 1),
                                             skip_group_check=True)
                    oT.append(pp)
                o3T = [pso.tile([128, 256], F32, name=f"o3t{i}{hp}", tag="o3t")
                       for hp in range(2)]
                for q in range(128):
                    u, qq = q // 4, q % 4
                    for hh in range(2):
                        for kh in range(2):
                            nc.tensor.matmul(o3T[hh][:, 2 * q:2 * q + 2],
                                             tmv_chunk(u, qq, kh, hh),
                                             ATp[hh][kh][:, 2 * q:2 * q + 2],
                                             start=(kh == 0), stop=(kh == 1),
                                             skip_group_check=True)
                # combine (diagonal extract) + transpose to token-major
                ops = psp.tile([128, 256], F32, name=f"ops{i}", tag="proj")
                for hp in range(2):
                    o12sb = rp.tile([128, 128], F32, name=f"o12sb{i}{hp}", tag="o12sb")
                    nc.scalar.activation(o12sb[:], oT[hp][:], AF.Copy)
                    osb = rp.tile([128, 128], F32, name=f"osb{i}{hp}", tag="osb")
                    for e in range(2):
                        nc.vector.tensor_add(osb[64 * e:64 * (e + 1), :],
                                             o3T[hp][64 * e:64 * (e + 1), e:256:2],
                                             o12sb[64 * e:64 * (e + 1), :])
                    nc.tensor.transpose(ops[:, 128 * hp:128 * (hp + 1)], osb[:], ident)

                # residual add with per-head softmax normalization folded in
                seqs1 = acp.tile([128, 256], F32, name=f"seqs1_{i}", tag=f"seqs1_{i}")
                for h in range(4):
                    sl = slice(64 * h, 64 * (h + 1))
                    nc.vector.scalar_tensor_tensor(
                        out=seqs1[:, sl], in0=ops[:, sl], scalar=rr[h][:],
                        in1=seqs_cur[:, sl], op0=OP.mult, op1=OP.add)

                # ---- FFN ----
                rs2 = rmsnorm_rs(i, seqs1, "f")
                h2 = rp.tile([128, 256], F32, name=f"h2_{i}", tag="x")
                nc.vector.tensor_scalar_mul(h2[:], seqs1[:], rs2[:])
                h2T = transpose_pair(i, h2, "h", BF)
                f1 = []
                for t in range(2):
                    pp = psp.tile([128, 128], F32, name=f"f1ps{i}{t}", tag="proj")
                    for a in range(2):
                        nc.tensor.matmul(pp[:], w1(i, a)[:, 128 * t:128 * (t + 1)],
                                         h2T[a][:], start=(a == 0), stop=(a == 1))
                    ft = acp.tile([128, 128], BF, name=f"f1_{i}{t}", tag=f"f1_{i}{t}")
                    nc.scalar.activation(ft[:], pp[:], AF.Relu, bias=b1(i, t))
                    f1.append(ft)
                f2 = []
                for t in range(2):
                    pp = psp.tile([128, 128], F32, name=f"f2ps{i}{t}", tag="proj")
                    for a in range(2):
                        nc.tensor.matmul(pp[:], w2(i, a)[:, 128 * t:128 * (t + 1)],
                                         f1[a][:], start=(a == 0), stop=(a == 1))
                    ft = acp.tile([128, 128], F32, name=f"f2_{i}{t}", tag=f"f2_{i}{t}")
                    nc.scalar.activation(ft[:], pp[:], AF.Identity, bias=b2(i, t))
                    f2.append(ft)
                ftok = psp.tile([128, 256], F32, name=f"ftok{i}", tag="proj")
                for t in range(2):
                    nc.tensor.transpose(ftok[:, 128 * t:128 * (t + 1)], f2[t][:], ident)
                seqs2 = acp.tile([128, 256], F32, name=f"seqs2_{i}", tag=f"seqs2_{i}")
                nc.vector.tensor_add(seqs2[:], seqs1[:], ftok[:])
                nc.vector.tensor_scalar_mul(seqs2[:], seqs2[:], npad)
                seqs_cur = seqs2

                if i == 0:
                    # exchange updated halves within the core pair; the PE
                    # transposes that consume hf are deferred into block 2 so
                    # block-2 w3 matmuls run during the collective
                    gin = dpool.tile([128, 256], F32, name="gin", tag="gin")
                    gout = dpool.tile([2, 128, 256], F32, name="gout", tag="gout")
                    nc.sync.dma_start(out=gin[:], in_=seqs2[:])
                    nc.gpsimd.collective_compute(
                        "AllGather", OP.bypass, replica_groups=AG_GROUPS,
                        ins=[gin.opt()], outs=[gout.opt()])
                    hf = []
                    for half in range(2):
                        t = acp.tile([128, 256], F32, name=f"hf{half}", tag=f"hf{half}")
                        nc.sync.dma_start(out=t[:], in_=gout[half])
                        hf.append(t)

            # ---- final rmsnorm ----
            rs3 = rmsnorm_rs(2, seqs_cur, "o")
            of = acp.tile([128, 256], F32, name="of", tag="of")
            nc.vector.tensor_scalar_mul(of[:], seqs_cur[:], rs3[:])
            nc.vector.tensor_mul(of[:], of[:], lnl)
            nc.sync.dma_start(out=d_out, in_=of[:])

    nc.compile()
    return nc


_CACHE = {}


def _get_nc():
    if "nc" not in _CACHE:
        _CACHE["nc"] = _build()
    return _CACHE["nc"]


def _run(inputs, trace=False, tmpdir=None):
    in_maps = [_prep_core(inputs, c) for c in range(NC)]
    nc = _get_nc()
    res = run_bass_kernel_spmd(nc, in_maps, list(range(NC)), trace=trace, tmpdir=tmpdir)
    out = np.zeros((B, L, H), np.float32)
    for c in range(NC):
        out[c // 2, (c % 2) * 128:(c % 2) * 128 + 128, :] = np.asarray(
            res.results[c]["out"], np.float32)
    return out, res


def kernel(**inputs):
    out, _ = _run(inputs, trace=False)
    return out


# revision 21
# speedup vs baseline: 1.0785x; 1.0785x over previous
"""AttentionDecoder Trainium2 Bass kernel.

Shapes (hardcoded): B=4, L=256, H=256, HEADS=4, D=64, BLOCKS=2.

Sharding: 8 cores; core c owns (batch b = c//2, query-half qh = c%2), i.e.
128 query rows x all 4 heads. Params replicated. Time matrices are sharded
on (batch, query) and streamed through the PE as the *stationary* operand
(fast-weight-load path), producing transposed logits w^T[k,q] / outputs
o^T[hd,q] whose per-query results land in PSUM *columns* (PE outputs must
start at 32-aligned partitions, so per-query row writes are not allowed).
An AllGather over core pairs exchanges updated activations between blocks;
the tmK matmuls of block 2 only need block-2 Q and the SBUF-resident time
matrices, so they execute during the exchange.

Host-side prep (free): layout transposes, folding ln/softmax-scale into
weights, casting the time-matrix stream to fp8, packing all small consts
into two DMA-able panels.
"""
import os
import sys

import numpy as np
import ml_dtypes

for _p in ("/opt/trn_rl_repo", os.path.expanduser("~/.axon_site/_ro/trn_rl_repo")):
    if os.path.isdir(_p) and _p not in sys.path:
        sys.path.insert(0, _p)
        break

import concourse.bacc as bacc
import concourse.mybir as mybir
import concourse.tile as tile
from concourse.bass_utils import run_bass_kernel_spmd

B, L, H, HEADS, BLOCKS = 4, 256, 256, 4, 2
D = 64
NC = 8
EPS = 1e-8
NEG = -4294967295.0
SCALE = 0.125

TM_FP8 = True  # time-matrix stream dtype: fp8e4m3 / bf16

F32 = mybir.dt.float32
BF = mybir.dt.bfloat16
TMDT = mybir.dt.float8e4 if TM_FP8 else BF
NPBF = ml_dtypes.bfloat16
TM_NP = ml_dtypes.float8_e4m3fn if TM_FP8 else NPBF
AF = mybir.ActivationFunctionType
OP = mybir.AluOpType
AG_GROUPS = [[0, 1], [2, 3], [4, 5], [6, 7]]

# f32 const-panel column offsets
MNEG_C, NPAD_C, LNL_C, QB_C, B1_C, B2_C, EPS_C, ID_C = 0, 256, 257, 513, 517, 521, 525, 526
F32P_COLS = 654
# bf16 const-panel column offsets
QW_C, KW_C, VW_C, W1_C, W2_C, APK_C, APV_C, ONES_C = 0, 1024, 2048, 3072, 4096, 5120, 6144, 7168
BFP_COLS = 7169


def _f32(x):
    return np.ascontiguousarray(x, np.float32)


def _prep_core(inp, c):
    """Host-side layout prep for core c. Pure data movement + dtype casts."""
    b, qh = c // 2, c % 2
    qs = slice(qh * 128, qh * 128 + 128)
    m = {}
    m["seqs_tok"] = _f32(inp["seqs"][b, qs, :])
    m["seqsT"] = _f32(inp["seqs"][b]).T.reshape(2, 128, 256).astype(NPBF)
    # tmK chunks [128(2q x 64d), 128 k-half]: unit u=(h,oct), col (jj*2+kh)*128;
    # 4 units per 1MB macro row-contiguous DMA
    arr = _f32(inp["time_matrix_K"][b, qs])  # [128q,256k,256h]
    a4 = arr.reshape(64, 2, 256, 4, 64).transpose(3, 0, 1, 4, 2).reshape(4, 64, 128, 256)
    units = a4.reshape(4, 8, 8, 128, 2, 128).transpose(0, 1, 3, 2, 4, 5).reshape(32, 128, 2048)
    m["tmK"] = np.ascontiguousarray(
        units.reshape(8, 4, 128, 2048).transpose(0, 2, 1, 3).reshape(8, 128, 8192)
    ).astype(TM_NP)
    # tmV chunks [128 k-half, 128 hd-half]: unit u=q//4, col (qq*4+kh*2+hh)*128
    arr = _f32(inp["time_matrix_V"][b, qs])  # [128q,256k,256hd]
    u6 = arr.reshape(32, 4, 2, 128, 2, 128).transpose(0, 3, 1, 2, 4, 5).reshape(32, 128, 2048)
    m["tmV"] = np.ascontiguousarray(
        u6.reshape(8, 4, 128, 2048).transpose(0, 2, 1, 3).reshape(8, 128, 8192)
    ).astype(TM_NP)

    am = np.asarray(inp["attention_mask"], bool)
    tlm = np.asarray(inp["timeline_mask"], bool)
    mneg = _f32(np.where(tlm[b, qs][:, None] | am[qs, :], NEG, 0.0))
    mnegT = _f32(mneg.T.reshape(2, 128, 128))  # [kh, k, q]
    npad = _f32(1.0 - tlm[b, qs].astype(np.float32))[:, None]
    lnl = _f32(np.broadcast_to(inp["ln_last"], (128, 256)))

    f32_parts = [mnegT[0], mnegT[1], npad, lnl]
    qb_parts, b1_parts, b2_parts = [], [], []
    qw_parts, kw_parts, vw_parts, w1_parts, w2_parts, apk_parts, apv_parts = ([] for _ in range(7))
    for i in range(BLOCKS):
        qw_eff = (_f32(inp["Qw"][i]) * _f32(inp["ln_attn"][i])[None, :] * SCALE).T  # [hin,hcol]
        qw4 = qw_eff.reshape(2, 128, 2, 128).transpose(0, 2, 1, 3)  # [a,t,128,128]
        for a in range(2):
            for t in range(2):
                qw_parts.append(qw4[a, t])
        for t in range(2):
            qb_parts.append(_f32(inp["Qb"][i] * SCALE).reshape(2, 128, 1)[t])
            b1_parts.append(_f32(inp["b1"][i]).reshape(2, 128, 1)[t])
            b2_parts.append(_f32(inp["b2"][i]).reshape(2, 128, 1)[t])
        kw_parts.extend(_f32(inp["Kw"][i]).T.reshape(2, 128, 256))
        vw_parts.extend(_f32(inp["Vw"][i]).T.reshape(2, 128, 256))
        w1_eff = (_f32(inp["W1"][i]) * _f32(inp["ln_ffn"][i])[None, :]).T
        w1_parts.extend(w1_eff.reshape(2, 128, 256))
        w2_parts.extend(_f32(inp["W2"][i]).T.reshape(2, 128, 256))
        apk_parts.extend((_f32(inp["abs_pos_K"][b]).T + _f32(inp["Kb"][i])[:, None]).reshape(2, 128, 256))
        apv_parts.extend((_f32(inp["abs_pos_V"][b]) + _f32(inp["Vb"][i])[None, :]).reshape(2, 128, 256))
    f32_parts += qb_parts + b1_parts + b2_parts
    f32_parts.append(np.full((128, 1), EPS, np.float32))
    f32_parts.append(_f32(np.eye(128)))
    f32p = np.concatenate(f32_parts, axis=1)
    assert f32p.shape == (128, F32P_COLS), f32p.shape
    m["f32p"] = _f32(f32p)
    bf_parts = (qw_parts + kw_parts + vw_parts + w1_parts + w2_parts +
                apk_parts + apv_parts + [np.ones((128, 1), np.float32)])
    bfp = np.concatenate(bf_parts, axis=1)
    assert bfp.shape == (128, BFP_COLS), bfp.shape
    m["bfp"] = bfp.astype(NPBF)
    return m


def _build():
    nc = bacc.Bacc("TRN2", target_bir_lowering=False, debug=False, num_devices=NC)

    def dp(name, shape, dt):
        return nc.dram_tensor(name, list(shape), dt, kind="ExternalInput").ap()

    d_seqs = dp("seqs_tok", (128, 256), F32)
    d_seqsT = dp("seqsT", (2, 128, 256), BF)
    d_f32p = dp("f32p", (128, F32P_COLS), F32)
    d_bfp = dp("bfp", (128, BFP_COLS), BF)
    d_tmK = dp("tmK", (8, 128, 8192), TMDT)
    d_tmV = dp("tmV", (8, 128, 8192), TMDT)
    d_out = nc.dram_tensor("out", [128, 256], F32, kind="ExternalOutput").ap()

    with tile.TileContext(nc) as tc:
        with (
            tc.tile_pool(name="wts", bufs=1) as wp,
            tc.tile_pool(name="act", bufs=1) as acp,
            tc.tile_pool(name="rot", bufs=2) as rp,
            tc.tile_pool(name="tmres", bufs=1) as tres,
            tc.tile_pool(name="psw", bufs=2, space="PSUM") as psw,
            tc.tile_pool(name="pso", bufs=2, space="PSUM") as pso,
            tc.tile_pool(name="pst", bufs=2, space="PSUM") as pst,
            tc.tile_pool(name="psp", bufs=2, space="PSUM") as psp,
            tc.tile_pool(name="dram", bufs=1, space="DRAM") as dpool,
        ):
            # activations + const panels first — nothing queues behind the
            # 16.8MB time-matrix burst
            seqs_cur = acp.tile([128, 256], F32, name="seqs0", tag="seqs0")
            nc.sync.dma_start(out=seqs_cur[:], in_=d_seqs)
            sT = []
            for a in range(2):
                t = acp.tile([128, 256], BF, name=f"sT0_{a}", tag=f"sT0_{a}")
                nc.sync.dma_start(out=t[:], in_=d_seqsT[a])
                sT.append(t)
            f32p = wp.tile([128, F32P_COLS], F32, name="f32p", tag="f32p")
            nc.sync.dma_start(out=f32p[:], in_=d_f32p)
            bfp = wp.tile([128, BFP_COLS], BF, name="bfp", tag="bfp")
            nc.sync.dma_start(out=bfp[:], in_=d_bfp)

            mnegT = lambda kh: f32p[:, MNEG_C + 128 * kh:MNEG_C + 128 * (kh + 1)]
            npad = f32p[:, NPAD_C:NPAD_C + 1]
            lnl = f32p[:, LNL_C:LNL_C + 256]
            qb = lambda i, t: f32p[:, QB_C + 2 * i + t:QB_C + 2 * i + t + 1]
            b1 = lambda i, t: f32p[:, B1_C + 2 * i + t:B1_C + 2 * i + t + 1]
            b2 = lambda i, t: f32p[:, B2_C + 2 * i + t:B2_C + 2 * i + t + 1]
            epsb = f32p[:, EPS_C:EPS_C + 1]
            ident = f32p[:, ID_C:ID_C + 128]
            qw = lambda i, a, t: bfp[:, QW_C + ((i * 2 + a) * 2 + t) * 128:
                                     QW_C + ((i * 2 + a) * 2 + t + 1) * 128]
            kw = lambda i, a: bfp[:, KW_C + (i * 2 + a) * 256:KW_C + (i * 2 + a + 1) * 256]
            vw = lambda i, a: bfp[:, VW_C + (i * 2 + a) * 256:VW_C + (i * 2 + a + 1) * 256]
            w1 = lambda i, a: bfp[:, W1_C + (i * 2 + a) * 256:W1_C + (i * 2 + a + 1) * 256]
            w2 = lambda i, a: bfp[:, W2_C + (i * 2 + a) * 256:W2_C + (i * 2 + a + 1) * 256]
            apk = lambda i, t: bfp[:, APK_C + (i * 2 + t) * 256:APK_C + (i * 2 + t + 1) * 256]
            apv = lambda i, t: bfp[:, APV_C + (i * 2 + t) * 256:APV_C + (i * 2 + t + 1) * 256]
            ones = bfp[:, ONES_C:ONES_C + 1]

            # resident tm macros (4 units each, loaded once, both blocks use
            # them); issue stays on sync so no compute engine stalls on DMA
            # queue backpressure
            tmk_res, tmv_res = [], []
            for mi in range(8):
                t = tres.tile([128, 8192], TMDT, name=f"rk{mi}", tag=f"rk{mi}")
                nc.sync.dma_start(out=t[:], in_=d_tmK[mi])
                tmk_res.append(t)
            for mi in range(8):
                t = tres.tile([128, 8192], TMDT, name=f"rv{mi}", tag=f"rv{mi}")
                nc.sync.dma_start(out=t[:], in_=d_tmV[mi])
                tmv_res.append(t)

            def tmk_chunk(u, jj, kh):
                c0 = (u % 4) * 2048 + (jj * 2 + kh) * 128
                return tmk_res[u // 4][:, c0:c0 + 128]

            def tmv_chunk(u, qq, kh, hh):
                c0 = (u % 4) * 2048 + (qq * 4 + kh * 2 + hh) * 128
                return tmv_res[u // 4][:, c0:c0 + 128]

            def rmsnorm_rs(i, src, label):
                """[128,1] f32 tile holding 1/sqrt(mean(src^2)+EPS)."""
                scr = rp.tile([128, 256], F32, name=f"scr_{label}{i}", tag="scr")
                ssum = rp.tile([128, 1], F32, name=f"ss_{label}{i}", tag="ss")
                nc.scalar.activation(scr[:], src[:], AF.Square, accum_out=ssum[:])
                st_ = rp.tile([128, 1], F32, name=f"st_{label}{i}", tag="st")
                nc.scalar.activation(st_[:], ssum[:], AF.Sqrt, scale=1.0 / 256.0,
                                     bias=epsb)
                rs_ = rp.tile([128, 1], F32, name=f"rs_{label}{i}", tag="rs")
                nc.vector.reciprocal(rs_[:], st_[:])
                return rs_

            def transpose_pair(i, src, label, out_dt):
                """[128,256] f32 -> two [128,128] out_dt transposed tiles."""
                outs = []
                for a in range(2):
                    tp = pst.tile([128, 128], F32, name=f"tp_{label}{i}{a}", tag="tr")
                    nc.tensor.transpose(tp[:], src[:, 128 * a:128 * (a + 1)], ident)
                    ot = rp.tile([128, 128], out_dt, name=f"{label}T{i}{a}", tag=f"{label}T{a}")
                    nc.vector.tensor_copy(ot[:], tp[:])
                    outs.append(ot)
                return outs

            hf = None
            for i in range(BLOCKS):
                # ---- rmsnorm (Q path) + Q projection (local data only) ----
                rs_ = rmsnorm_rs(i, seqs_cur, "q")
                x_sb = rp.tile([128, 256], F32, name=f"x{i}", tag="x")
                nc.vector.tensor_scalar_mul(x_sb[:], seqs_cur[:], rs_[:])
                xT = transpose_pair(i, x_sb, "x", BF)
                QT = []
                for t in range(2):
                    pp = psp.tile([128, 128], F32, name=f"qps{i}{t}", tag="proj")
                    for a in range(2):
                        nc.tensor.matmul(pp[:], qw(i, a, t), xT[a][:],
                                         start=(a == 0), stop=(a == 1))
                    qt = acp.tile([128, 128], BF, name=f"QT{i}{t}", tag=f"QT{i}{t}")
                    nc.scalar.activation(qt[:], pp[:], AF.Identity, bias=qb(i, t))
                    QT.append(qt)
                # block-diagonal packed Q (moving operand for the tmK matvecs)
                QS = []
                for h in range(4):
                    q = acp.tile([128, 128], TMDT, name=f"QS{i}{h}", tag=f"QS{i}{h}")
                    nc.vector.memset(q[:], 0.0)
                    src = QT[h // 2][64 * (h % 2):64 * (h % 2) + 64, :]
                    nc.vector.tensor_copy(q[0:64, 0:128:2], src[:, 0:128:2])
                    nc.vector.tensor_copy(q[64:128, 1:128:2], src[:, 1:128:2])
                    QS.append(q)

                # ---- w3: A . tmK, all heads, packed [128k, (e,kh)*128+2q] ----
                # needs only Q + resident tm: block 2's run during the
                # inter-block exchange
                wTb = [psw.tile([128, 512], F32, name=f"wb{i}{hp}", tag="w")
                       for hp in range(2)]
                for h in range(4):
                    hp, e = h // 2, h % 2
                    for kh in range(2):
                        cb = (e * 2 + kh) * 128
                        for j in range(64):
                            u, jj = h * 8 + j // 8, j % 8
                            nc.tensor.matmul(wTb[hp][:, cb + 2 * j:cb + 2 * j + 2],
                                             tmk_chunk(u, jj, kh),
                                             QS[h][:, 2 * j:2 * j + 2],
                                             start=True, stop=True,
                                             skip_group_check=True)

                # ---- K/V projections (block 2: needs exchanged seqsT) ----
                if i > 0:
                    sT = [acp.tile([128, 256], BF, name=f"sT{i}_{a}", tag=f"sT{i}_{a}")
                          for a in range(2)]
                    for a in range(2):
                        for half in range(2):
                            tp = pst.tile([128, 128], F32, name=f"tpe{i}{a}{half}", tag="tr")
                            nc.tensor.transpose(tp[:], hf[half][:, 128 * a:128 * (a + 1)],
                                                ident)
                            nc.vector.tensor_copy(sT[a][:, 128 * half:128 * (half + 1)], tp[:])
                KapT = []
                for t in range(2):
                    pp = psp.tile([128, 256], F32, name=f"kps{i}{t}", tag="proj")
                    for a in range(2):
                        nc.tensor.matmul(pp[:], kw(i, a)[:, 128 * t:128 * (t + 1)],
                                         sT[a][:], start=(a == 0), stop=(a == 1))
                    kt = acp.tile([128, 256], BF, name=f"KapT{i}{t}", tag=f"KapT{i}{t}")
                    nc.vector.tensor_add(kt[:], pp[:], apk(i, t))
                    KapT.append(kt)
                VapV = []
                for t in range(2):
                    pp = psp.tile([128, 256], F32, name=f"vps{i}{t}", tag="proj")
                    for a in range(2):
                        nc.tensor.matmul(pp[:], sT[a][:, 128 * t:128 * (t + 1)],
                                         vw(i, a), start=(a == 0), stop=(a == 1))
                    vt = acp.tile([128, 256], BF, name=f"VapV{i}{t}", tag=f"VapV{i}{t}")
                    nc.vector.tensor_add(vt[:], pp[:], apv(i, t))
                    VapV.append(vt)

                # ---- w12 + mask + softmax (transposed layout) ----
                ATp = [[acp.tile([128, 256], TMDT, name=f"ATp{i}{hp}{kh}", tag=f"ATp{i}{hp}{kh}")
                        for kh in range(2)] for hp in range(2)]
                ATn = [[acp.tile([128, 128], BF, name=f"ATn{i}{h}{kh}", tag=f"ATn{i}{h}{kh}")
                        for kh in range(2)] for h in range(4)]
                sps = pst.tile([128, 4], F32, name=f"sps{i}", tag="tr")
                for h in range(4):
                    hp, e = h // 2, h % 2
                    for kh in range(2):
                        w12 = psp.tile([128, 128], F32, name=f"w12_{i}{h}{kh}", tag="proj")
                        lhs = KapT[h // 2][64 * (h % 2):64 * (h % 2) + 64,
                                           128 * kh:128 * (kh + 1)]
                        rhs = QT[h // 2][64 * (h % 2):64 * (h % 2) + 64, :]
                        nc.tensor.matmul(w12[:], lhs, rhs, start=True, stop=True)
                        wmk = rp.tile([128, 128], F32, name=f"wmk{i}{h}{kh}", tag="wmk")
                        nc.vector.tensor_add(wmk[:], w12[:], mnegT(kh))
                        wm = rp.tile([128, 128], F32, name=f"wm{i}{h}{kh}", tag="wm")
                        nc.vector.tensor_add(
                            wm[:], wTb[hp][:, (e * 2 + kh) * 128:(e * 2 + kh + 1) * 128],
                            wmk[:])
                        nc.scalar.activation(ATp[hp][kh][:, e:256:2], wm[:], AF.Exp)
                        nc.scalar.activation(ATn[h][kh][:], wm[:], AF.Exp)
                        # per-query row sums: sps[:, h] += ATn^T @ ones
                        nc.tensor.matmul(sps[:, h:h + 1], ATn[h][kh][:], ones,
                                         start=(kh == 0), stop=(kh == 1),
                                         skip_group_check=True)
                rr = []
                for h in range(4):
                    r_h = acp.tile([128, 1], F32, name=f"r{i}{h}", tag=f"r{i}{h}")
                    nc.vector.reciprocal(r_h[:], sps[:, h:h + 1])
                    rr.append(r_h)

                # ---- attention output (transposed accumulation) ----
                oT = []
                for hp in range(2):
                    pp = psp.tile([128, 128], F32, name=f"o12t{i}{hp}", tag="proj")
                    for e in range(2):
                        h = 2 * hp + e
                        for kh in range(2):
                            nc.tensor.matmul(pp[64 * e:64 * (e + 1), :],
                                             VapV[kh][:, 64 * h:64 * (h + 1)],
                                             ATn[h][kh][:],
                                             start=(kh == 0), stop=(kh == 1),
                                             skip_group_check=True)
                    oT.append(pp)
                o3T = [pso.tile([128, 256], F32, name=f"o3t{i}{hp}", tag="o3t")
                       for hp in range(2)]
                for q in range(128):
                    u, qq = q // 4, q % 4
                    for hh in range(2):
                        for kh in range(2):
                            nc.tensor.matmul(o3T[hh][:, 2 * q:2 * q + 2],
                                             tmv_chunk(u, qq, kh, hh),
                                             ATp[hh][kh][:, 2 * q:2 * q + 2],
                                             start=(kh == 0), stop=(kh == 1),
                                             skip_group_check=True)
                # combine (diagonal extract) + transpose to token-major
                ops = psp.tile([128, 256], F32, name=f"ops{i}", tag="proj")
                for hp in range(2):
                    o12sb = rp.tile([128, 128], F32, name=f"o12sb{i}{hp}", tag="o12sb")
                    nc.scalar.activation(o12sb[:], oT[hp][:], AF.Copy)
                    osb = rp.tile([128, 128], F32, name=f"osb{i}{hp}", tag="osb")
                    for e in range(2):
                        nc.vector.tensor_add(osb[64 * e:64 * (e + 1), :],
                                             o3T[hp][64 * e:64 * (e + 1), e:256:2],
                                             o12sb[64 * e:64 * (e + 1), :])
                    nc.tensor.transpose(ops[:, 128 * hp:128 * (hp + 1)], osb[:], ident)

                # residual add with per-head softmax normalization folded in
                seqs1 = acp.tile([128, 256], F32, name=f"seqs1_{i}", tag=f"seqs1_{i}")
                for h in range(4):
                    sl = slice(64 * h, 64 * (h + 1))
                    nc.vector.scalar_tensor_tensor(
                        out=seqs1[:, sl], in0=ops[:, sl], scalar=rr[h][:],
                        in1=seqs_cur[:, sl], op0=OP.mult, op1=OP.add)

                # ---- FFN ----
                rs2 = rmsnorm_rs(i, seqs1, "f")
                h2 = rp.tile([128, 256], F32, name=f"h2_{i}", tag="x")
                nc.vector.tensor_scalar_mul(h2[:], seqs1[:], rs2[:])
                h2T = transpose_pair(i, h2, "h", BF)
                f1 = []
                for t in range(2):
                    pp = psp.tile([128, 128], F32, name=f"f1ps{i}{t}", tag="proj")
                    for a in range(2):
                        nc.tensor.matmul(pp[:], w1(i, a)[:, 128 * t:128 * (t + 1)],
                                         h2T[a][:], start=(a == 0), stop=(a == 1))
                    ft = acp.tile([128, 128], BF, name=f"f1_{i}{t}", tag=f"f1_{i}{t}")
                    nc.scalar.activation(ft[:], pp[:], AF.Relu, bias=b1(i, t))
                    f1.append(ft)
                f2 = []
                for t in range(2):
                    pp = psp.tile([128, 128], F32, name=f"f2ps{i}{t}", tag="proj")
                    for a in range(2):
                        nc.tensor.matmul(pp[:], w2(i, a)[:, 128 * t:128 * (t + 1)],
                                         f1[a][:], start=(a == 0), stop=(a == 1))
                    ft = acp.tile([128, 128], F32, name=f"f2_{i}{t}", tag=f"f2_{i}{t}")
                    nc.scalar.activation(ft[:], pp[:], AF.Identity, bias=b2(i, t))
                    f2.append(ft)
                ftok = psp.tile([128, 256], F32, name=f"ftok{i}", tag="proj")
                for t in range(2):
                    nc.tensor.transpose(ftok[:, 128 * t:128 * (t + 1)], f2[t][:], ident)
                seqs2 = acp.tile([128, 256], F32, name=f"seqs2_{i}", tag=f"seqs2_{i}")
                nc.vector.tensor_add(seqs2[:], seqs1[:], ftok[:])
                nc.vector.tensor_scalar_mul(seqs2[:], seqs2[:], npad)
                seqs_cur = seqs2

                if i == 0:
                    # exchange updated halves within the core pair; the PE
                    # transposes that consume hf are deferred into block 2 so
                    # block-2 w3 matmuls run during the collective
                    gin = dpool.tile([128, 256], F32, name="gin", tag="gin")
                    gout = dpool.tile([2, 128, 256], F32, name="gout", tag="gout")
                    nc.sync.dma_start(out=gin[:], in_=seqs2[:])
                    nc.gpsimd.collective_compute(
                        "AllGather", OP.bypass, replica_groups=AG_GROUPS,
                        ins=[gin.opt()], outs=[gout.opt()])
                    hf = []
                    for half in range(2):
                        t = acp.tile([128, 256], F32, name=f"hf{half}", tag=f"hf{half}")
                        nc.sync.dma_start(out=t[:], in_=gout[half])
                        hf.append(t)

            # ---- final rmsnorm ----
            rs3 = rmsnorm_rs(2, seqs_cur, "o")
            of = acp.tile([128, 256], F32, name="of", tag="of")
            nc.vector.tensor_scalar_mul(of[:], seqs_cur[:], rs3[:])
            nc.vector.tensor_mul(of[:], of[:], lnl)
            nc.sync.dma_start(out=d_out, in_=of[:])

    nc.compile()
    return nc


_CACHE = {}


def _get_nc():
    if "nc" not in _CACHE:
        _CACHE["nc"] = _build()
    return _CACHE["nc"]


def _run(inputs, trace=False, tmpdir=None):
    in_maps = [_prep_core(inputs, c) for c in range(NC)]
    nc = _get_nc()
    res = run_bass_kernel_spmd(nc, in_maps, list(range(NC)), trace=trace, tmpdir=tmpdir)
    out = np.zeros((B, L, H), np.float32)
    for c in range(NC):
        out[c // 2, (c % 2) * 128:(c % 2) * 128 + 128, :] = np.asarray(
            res.results[c]["out"], np.float32)
    return out, res


def kernel(**inputs):
    out, _ = _run(inputs, trace=False)
    return out


# revision 23
# speedup vs baseline: 1.1588x; 1.0744x over previous
"""AttentionDecoder Trainium2 Bass kernel.

Shapes (hardcoded): B=4, L=256, H=256, HEADS=4, D=64, BLOCKS=2.

Sharding: 8 cores; core c owns (batch b = c//2, query-half qh = c%2), i.e.
128 query rows x all 4 heads. Params replicated. Time matrices are sharded
on (batch, query) and streamed through the PE as the *stationary* operand
(fast-weight-load path), producing transposed logits w^T[k,q] / outputs
o^T[hd,q] whose per-query results land in PSUM *columns* (PE outputs must
start at 32-aligned partitions, so per-query row writes are not allowed).
An AllGather over core pairs exchanges updated activations between blocks;
the tmK matmuls of block 2 only need block-2 Q and the SBUF-resident time
matrices, so they execute during the exchange.

Host-side prep (free): layout transposes, folding ln/softmax-scale into
weights, casting the time-matrix stream to fp8, packing all small consts
into two DMA-able panels.
"""
import os
import sys

import numpy as np
import ml_dtypes

for _p in ("/opt/trn_rl_repo", os.path.expanduser("~/.axon_site/_ro/trn_rl_repo")):
    if os.path.isdir(_p) and _p not in sys.path:
        sys.path.insert(0, _p)
        break

import concourse.bacc as bacc
import concourse.mybir as mybir
import concourse.tile as tile
from concourse.bass_utils import run_bass_kernel_spmd

B, L, H, HEADS, BLOCKS = 4, 256, 256, 4, 2
D = 64
NC = 8
EPS = 1e-8
NEG = -4294967295.0
SCALE = 0.125

TM_FP8 = True  # time-matrix stream dtype: fp8e4m3 / bf16

F32 = mybir.dt.float32
BF = mybir.dt.bfloat16
TMDT = mybir.dt.float8e4 if TM_FP8 else BF
NPBF = ml_dtypes.bfloat16
TM_NP = ml_dtypes.float8_e4m3fn if TM_FP8 else NPBF
AF = mybir.ActivationFunctionType
OP = mybir.AluOpType
AG_GROUPS = [[0, 1], [2, 3], [4, 5], [6, 7]]

# f32 const-panel column offsets
MNEG_C, NPAD_C, LNL_C, QB_C, B1_C, B2_C, EPS_C, ID_C = 0, 256, 257, 513, 517, 521, 525, 526
F32P_COLS = 654
# bf16 const-panel column offsets
QW_C, KW_C, VW_C, W1_C, W2_C, APK_C, APV_C, ONES_C = 0, 1024, 2048, 3072, 4096, 5120, 6144, 7168
BFP_COLS = 7169


def _f32(x):
    return np.ascontiguousarray(x, np.float32)


def _prep_core(inp, c):
    """Host-side layout prep for core c. Pure data movement + dtype casts."""
    b, qh = c // 2, c % 2
    qs = slice(qh * 128, qh * 128 + 128)
    m = {}
    m["seqs_tok"] = _f32(inp["seqs"][b, qs, :])
    m["seqsT"] = _f32(inp["seqs"][b]).T.reshape(2, 128, 256).astype(NPBF)
    # tmK chunks [128(2q x 64d), 128 k-half]: unit u=(h,oct), col (jj*2+kh)*128;
    # 4 units per 1MB macro row-contiguous DMA
    arr = _f32(inp["time_matrix_K"][b, qs])  # [128q,256k,256h]
    a4 = arr.reshape(64, 2, 256, 4, 64).transpose(3, 0, 1, 4, 2).reshape(4, 64, 128, 256)
    units = a4.reshape(4, 8, 8, 128, 2, 128).transpose(0, 1, 3, 2, 4, 5).reshape(32, 128, 2048)
    m["tmK"] = np.ascontiguousarray(
        units.reshape(8, 4, 128, 2048).transpose(0, 2, 1, 3).reshape(8, 128, 8192)
    ).astype(TM_NP)
    # tmV chunks [128 k-half, 128 hd-half]: unit u=q//4, col (qq*4+kh*2+hh)*128
    arr = _f32(inp["time_matrix_V"][b, qs])  # [128q,256k,256hd]
    u6 = arr.reshape(32, 4, 2, 128, 2, 128).transpose(0, 3, 1, 2, 4, 5).reshape(32, 128, 2048)
    m["tmV"] = np.ascontiguousarray(
        u6.reshape(8, 4, 128, 2048).transpose(0, 2, 1, 3).reshape(8, 128, 8192)
    ).astype(TM_NP)

    am = np.asarray(inp["attention_mask"], bool)
    tlm = np.asarray(inp["timeline_mask"], bool)
    mneg = _f32(np.where(tlm[b, qs][:, None] | am[qs, :], NEG, 0.0))
    mnegT = _f32(mneg.T.reshape(2, 128, 128))  # [kh, k, q]
    npad = _f32(1.0 - tlm[b, qs].astype(np.float32))[:, None]
    lnl = _f32(np.broadcast_to(inp["ln_last"], (128, 256)))

    f32_parts = [mnegT[0], mnegT[1], npad, lnl]
    qb_parts, b1_parts, b2_parts = [], [], []
    qw_parts, kw_parts, vw_parts, w1_parts, w2_parts, apk_parts, apv_parts = ([] for _ in range(7))
    for i in range(BLOCKS):
        qw_eff = (_f32(inp["Qw"][i]) * _f32(inp["ln_attn"][i])[None, :] * SCALE).T  # [hin,hcol]
        qw4 = qw_eff.reshape(2, 128, 2, 128).transpose(0, 2, 1, 3)  # [a,t,128,128]
        for a in range(2):
            for t in range(2):
                qw_parts.append(qw4[a, t])
        for t in range(2):
            qb_parts.append(_f32(inp["Qb"][i] * SCALE).reshape(2, 128, 1)[t])
            b1_parts.append(_f32(inp["b1"][i]).reshape(2, 128, 1)[t])
            b2_parts.append(_f32(inp["b2"][i]).reshape(2, 128, 1)[t])
        kw_parts.extend(_f32(inp["Kw"][i]).T.reshape(2, 128, 256))
        vw_parts.extend(_f32(inp["Vw"][i]).T.reshape(2, 128, 256))
        w1_eff = (_f32(inp["W1"][i]) * _f32(inp["ln_ffn"][i])[None, :]).T
        w1_parts.extend(w1_eff.reshape(2, 128, 256))
        w2_parts.extend(_f32(inp["W2"][i]).T.reshape(2, 128, 256))
        apk_parts.extend((_f32(inp["abs_pos_K"][b]).T + _f32(inp["Kb"][i])[:, None]).reshape(2, 128, 256))
        apv_parts.extend((_f32(inp["abs_pos_V"][b]) + _f32(inp["Vb"][i])[None, :]).reshape(2, 128, 256))
    f32_parts += qb_parts + b1_parts + b2_parts
    f32_parts.append(np.full((128, 1), EPS, np.float32))
    f32_parts.append(_f32(np.eye(128)))
    f32p = np.concatenate(f32_parts, axis=1)
    assert f32p.shape == (128, F32P_COLS), f32p.shape
    m["f32p"] = _f32(f32p)
    bf_parts = (qw_parts + kw_parts + vw_parts + w1_parts + w2_parts +
                apk_parts + apv_parts + [np.ones((128, 1), np.float32)])
    bfp = np.concatenate(bf_parts, axis=1)
    assert bfp.shape == (128, BFP_COLS), bfp.shape
    m["bfp"] = bfp.astype(NPBF)
    return m


def _build():
    nc = bacc.Bacc("TRN2", target_bir_lowering=False, debug=False, num_devices=NC)

    def dp(name, shape, dt):
        return nc.dram_tensor(name, list(shape), dt, kind="ExternalInput").ap()

    d_seqs = dp("seqs_tok", (128, 256), F32)
    d_seqsT = dp("seqsT", (2, 128, 256), BF)
    d_f32p = dp("f32p", (128, F32P_COLS), F32)
    d_bfp = dp("bfp", (128, BFP_COLS), BF)
    d_tmK = dp("tmK", (8, 128, 8192), TMDT)
    d_tmV = dp("tmV", (8, 128, 8192), TMDT)
    d_out = nc.dram_tensor("out", [128, 256], F32, kind="ExternalOutput").ap()

    with tile.TileContext(nc) as tc:
        with (
            tc.tile_pool(name="wts", bufs=1) as wp,
            tc.tile_pool(name="act", bufs=1) as acp,
            tc.tile_pool(name="rot", bufs=2) as rp,
            tc.tile_pool(name="tmres", bufs=1) as tres,
            tc.tile_pool(name="psw", bufs=2, space="PSUM") as psw,
            tc.tile_pool(name="pso", bufs=2, space="PSUM") as pso,
            tc.tile_pool(name="pst", bufs=2, space="PSUM") as pst,
            tc.tile_pool(name="psp", bufs=2, space="PSUM") as psp,
            tc.tile_pool(name="dram", bufs=1, space="DRAM") as dpool,
        ):
            # activations + const panels first — nothing queues behind the
            # 16.8MB time-matrix burst
            seqs_cur = acp.tile([128, 256], F32, name="seqs0", tag="seqs0")
            nc.sync.dma_start(out=seqs_cur[:], in_=d_seqs)
            sT = []
            for a in range(2):
                t = acp.tile([128, 256], BF, name=f"sT0_{a}", tag=f"sT0_{a}")
                nc.sync.dma_start(out=t[:], in_=d_seqsT[a])
                sT.append(t)
            f32p = wp.tile([128, F32P_COLS], F32, name="f32p", tag="f32p")
            nc.sync.dma_start(out=f32p[:], in_=d_f32p)
            bfp = wp.tile([128, BFP_COLS], BF, name="bfp", tag="bfp")
            nc.sync.dma_start(out=bfp[:], in_=d_bfp)

            mnegT = lambda kh: f32p[:, MNEG_C + 128 * kh:MNEG_C + 128 * (kh + 1)]
            npad = f32p[:, NPAD_C:NPAD_C + 1]
            lnl = f32p[:, LNL_C:LNL_C + 256]
            qb = lambda i, t: f32p[:, QB_C + 2 * i + t:QB_C + 2 * i + t + 1]
            b1 = lambda i, t: f32p[:, B1_C + 2 * i + t:B1_C + 2 * i + t + 1]
            b2 = lambda i, t: f32p[:, B2_C + 2 * i + t:B2_C + 2 * i + t + 1]
            epsb = f32p[:, EPS_C:EPS_C + 1]
            ident = f32p[:, ID_C:ID_C + 128]
            qw = lambda i, a, t: bfp[:, QW_C + ((i * 2 + a) * 2 + t) * 128:
                                     QW_C + ((i * 2 + a) * 2 + t + 1) * 128]
            kw = lambda i, a: bfp[:, KW_C + (i * 2 + a) * 256:KW_C + (i * 2 + a + 1) * 256]
            vw = lambda i, a: bfp[:, VW_C + (i * 2 + a) * 256:VW_C + (i * 2 + a + 1) * 256]
            w1 = lambda i, a: bfp[:, W1_C + (i * 2 + a) * 256:W1_C + (i * 2 + a + 1) * 256]
            w2 = lambda i, a: bfp[:, W2_C + (i * 2 + a) * 256:W2_C + (i * 2 + a + 1) * 256]
            apk = lambda i, t: bfp[:, APK_C + (i * 2 + t) * 256:APK_C + (i * 2 + t + 1) * 256]
            apv = lambda i, t: bfp[:, APV_C + (i * 2 + t) * 256:APV_C + (i * 2 + t + 1) * 256]
            ones = bfp[:, ONES_C:ONES_C + 1]

            # resident tm macros (4 units each, loaded once, both blocks use
            # them); issue stays on sync so no compute engine stalls on DMA
            # queue backpressure
            tmk_res, tmv_res = [None] * 8, [None] * 8
            # issue order matches block-1 head processing order 0,2,1,3
            # (macro pair 2m, 2m+1 belongs to head m)
            for hd in (0, 2, 1, 3):
                for half in range(2):
                    mi = 2 * hd + half
                    t = tres.tile([128, 8192], TMDT, name=f"rk{mi}", tag=f"rk{mi}")
                    nc.sync.dma_start(out=t[:], in_=d_tmK[mi])
                    tmk_res[mi] = t
            for mi in range(8):
                t = tres.tile([128, 8192], TMDT, name=f"rv{mi}", tag=f"rv{mi}")
                nc.sync.dma_start(out=t[:], in_=d_tmV[mi])
                tmv_res[mi] = t

            def tmk_chunk(u, jj, kh):
                c0 = (u % 4) * 2048 + (jj * 2 + kh) * 128
                return tmk_res[u // 4][:, c0:c0 + 128]

            def tmv_chunk(u, qq, kh, hh):
                c0 = (u % 4) * 2048 + (qq * 4 + kh * 2 + hh) * 128
                return tmv_res[u // 4][:, c0:c0 + 128]

            def rmsnorm_rs(i, src, label):
                """[128,1] f32 tile holding 1/sqrt(mean(src^2)+EPS)."""
                scr = rp.tile([128, 256], F32, name=f"scr_{label}{i}", tag="scr")
                ssum = rp.tile([128, 1], F32, name=f"ss_{label}{i}", tag="ss")
                nc.scalar.activation(scr[:], src[:], AF.Square, accum_out=ssum[:])
                st_ = rp.tile([128, 1], F32, name=f"st_{label}{i}", tag="st")
                nc.scalar.activation(st_[:], ssum[:], AF.Sqrt, scale=1.0 / 256.0,
                                     bias=epsb)
                rs_ = rp.tile([128, 1], F32, name=f"rs_{label}{i}", tag="rs")
                nc.vector.reciprocal(rs_[:], st_[:])
                return rs_

            def transpose_pair(i, src, label, out_dt):
                """[128,256] f32 -> two [128,128] out_dt transposed tiles."""
                outs = []
                for a in range(2):
                    tp = pst.tile([128, 128], F32, name=f"tp_{label}{i}{a}", tag="tr")
                    nc.tensor.transpose(tp[:], src[:, 128 * a:128 * (a + 1)], ident)
                    ot = rp.tile([128, 128], out_dt, name=f"{label}T{i}{a}", tag=f"{label}T{a}")
                    nc.vector.tensor_copy(ot[:], tp[:])
                    outs.append(ot)
                return outs

            hf = None
            for i in range(BLOCKS):
                # ---- rmsnorm (Q path) + Q projection (local data only) ----
                rs_ = rmsnorm_rs(i, seqs_cur, "q")
                x_sb = rp.tile([128, 256], F32, name=f"x{i}", tag="x")
                nc.vector.tensor_scalar_mul(x_sb[:], seqs_cur[:], rs_[:])
                xT = transpose_pair(i, x_sb, "x", BF)
                QT = []
                for t in range(2):
                    pp = psp.tile([128, 128], F32, name=f"qps{i}{t}", tag="proj")
                    for a in range(2):
                        nc.tensor.matmul(pp[:], qw(i, a, t), xT[a][:],
                                         start=(a == 0), stop=(a == 1))
                    qt = acp.tile([128, 128], BF, name=f"QT{i}{t}", tag=f"QT{i}{t}")
                    nc.scalar.activation(qt[:], pp[:], AF.Identity, bias=qb(i, t))
                    QT.append(qt)
                # block-diagonal packed Q (moving operand for the tmK matvecs)
                QS = []
                for h in range(4):
                    q = acp.tile([128, 128], TMDT, name=f"QS{i}{h}", tag=f"QS{i}{h}")
                    nc.vector.memset(q[:], 0.0)
                    src = QT[h // 2][64 * (h % 2):64 * (h % 2) + 64, :]
                    nc.vector.tensor_copy(q[0:64, 0:128:2], src[:, 0:128:2])
                    nc.vector.tensor_copy(q[64:128, 1:128:2], src[:, 1:128:2])
                    QS.append(q)

                # ---- attention logits: w3 (A . tmK) + w12, softmax ----
                wTb = [psw.tile([128, 512], F32, name=f"wb{i}{hp}", tag="w")
                       for hp in range(2)]
                ATp = [[acp.tile([128, 256], TMDT, name=f"ATp{i}{hp}{kh}", tag=f"ATp{i}{hp}{kh}")
                        for kh in range(2)] for hp in range(2)]
                ATn = [[acp.tile([128, 128], BF, name=f"ATn{i}{h}{kh}", tag=f"ATn{i}{h}{kh}")
                        for kh in range(2)] for h in range(4)]
                sps = pst.tile([128, 4], F32, name=f"sps{i}", tag="tr")
                KapT = [None, None]
                VapV = [None, None]

                def w3_head(h):
                    hp, e = h // 2, h % 2
                    for kh in range(2):
                        cb = (e * 2 + kh) * 128
                        for j in range(64):
                            u, jj = h * 8 + j // 8, j % 8
                            nc.tensor.matmul(wTb[hp][:, cb + 2 * j:cb + 2 * j + 2],
                                             tmk_chunk(u, jj, kh),
                                             QS[h][:, 2 * j:2 * j + 2],
                                             start=True, stop=True,
                                             skip_group_check=True)

                def kv_proj():
                    for t in range(2):
                        pp = psp.tile([128, 256], F32, name=f"kps{i}{t}", tag="proj")
                        for a in range(2):
                            nc.tensor.matmul(pp[:], kw(i, a)[:, 128 * t:128 * (t + 1)],
                                             sT[a][:], start=(a == 0), stop=(a == 1))
                        kt = acp.tile([128, 256], BF, name=f"KapT{i}{t}", tag=f"KapT{i}{t}")
                        nc.vector.tensor_add(kt[:], pp[:], apk(i, t))
                        KapT[t] = kt
                    for t in range(2):
                        pp = psp.tile([128, 256], F32, name=f"vps{i}{t}", tag="proj")
                        for a in range(2):
                            nc.tensor.matmul(pp[:], sT[a][:, 128 * t:128 * (t + 1)],
                                             vw(i, a), start=(a == 0), stop=(a == 1))
                        vt = acp.tile([128, 256], BF, name=f"VapV{i}{t}", tag=f"VapV{i}{t}")
                        nc.vector.tensor_add(vt[:], pp[:], apv(i, t))
                        VapV[t] = vt

                def softmax_head(h):
                    hp, e = h // 2, h % 2
                    for kh in range(2):
                        w12 = psp.tile([128, 128], F32, name=f"w12_{i}{h}{kh}", tag="proj")
                        lhs = KapT[h // 2][64 * (h % 2):64 * (h % 2) + 64,
                                           128 * kh:128 * (kh + 1)]
                        rhs = QT[h // 2][64 * (h % 2):64 * (h % 2) + 64, :]
                        nc.tensor.matmul(w12[:], lhs, rhs, start=True, stop=True)
                        wmk = rp.tile([128, 128], F32, name=f"wmk{i}{h}{kh}", tag="wmk")
                        nc.vector.tensor_add(wmk[:], w12[:], mnegT(kh))
                        wm = rp.tile([128, 128], F32, name=f"wm{i}{h}{kh}", tag="wm")
                        nc.vector.tensor_add(
                            wm[:], wTb[hp][:, (e * 2 + kh) * 128:(e * 2 + kh + 1) * 128],
                            wmk[:])
                        nc.scalar.activation(ATp[hp][kh][:, e:256:2], wm[:], AF.Exp)
                        nc.scalar.activation(ATn[h][kh][:], wm[:], AF.Exp)
                        # per-query row sums: sps[:, h] += ATn^T @ ones
                        nc.tensor.matmul(sps[:, h:h + 1], ATn[h][kh][:], ones,
                                         start=(kh == 0), stop=(kh == 1),
                                         skip_group_check=True)

                if i == 0:
                    # interleave per head (order avoids same-PSUM-bank WAR
                    # between a head's softmax read and its tile-mate's w3)
                    kv_proj()
                    for h in (0, 2, 1, 3):
                        w3_head(h)
                        softmax_head(h)
                else:
                    # all w3 first: only needs Q + resident tm, so it runs
                    # during the inter-block exchange; then rebuild seqsT
                    # from the gathered halves and do K/V + softmax
                    for h in (0, 2, 1, 3):
                        w3_head(h)
                    sT = [acp.tile([128, 256], BF, name=f"sT{i}_{a}", tag=f"sT{i}_{a}")
                          for a in range(2)]
                    for a in range(2):
                        for half in range(2):
                            tp = pst.tile([128, 128], F32, name=f"tpe{i}{a}{half}", tag="tr")
                            nc.tensor.transpose(tp[:], hf[half][:, 128 * a:128 * (a + 1)],
                                                ident)
                            nc.vector.tensor_copy(sT[a][:, 128 * half:128 * (half + 1)], tp[:])
                    kv_proj()
                    for h in range(4):
                        softmax_head(h)
                rr = []
                for h in range(4):
                    r_h = acp.tile([128, 1], F32, name=f"r{i}{h}", tag=f"r{i}{h}")
                    nc.vector.reciprocal(r_h[:], sps[:, h:h + 1])
                    rr.append(r_h)

                # ---- attention output (transposed accumulation) ----
                oT = []
                for hp in range(2):
                    pp = psp.tile([128, 128], F32, name=f"o12t{i}{hp}", tag="proj")
                    for e in range(2):
                        h = 2 * hp + e
                        for kh in range(2):
                            nc.tensor.matmul(pp[64 * e:64 * (e + 1), :],
                                             VapV[kh][:, 64 * h:64 * (h + 1)],
                                             ATn[h][kh][:],
                                             start=(kh == 0), stop=(kh == 1),
                                             skip_group_check=True)
                    oT.append(pp)
                o3T = [pso.tile([128, 256], F32, name=f"o3t{i}{hp}", tag="o3t")
                       for hp in range(2)]
                for q in range(128):
                    u, qq = q // 4, q % 4
                    for hh in range(2):
                        for kh in range(2):
                            nc.tensor.matmul(o3T[hh][:, 2 * q:2 * q + 2],
                                             tmv_chunk(u, qq, kh, hh),
                                             ATp[hh][kh][:, 2 * q:2 * q + 2],
                                             start=(kh == 0), stop=(kh == 1),
                                             skip_group_check=True)
                # combine (diagonal extract) + transpose to token-major
                ops = psp.tile([128, 256], F32, name=f"ops{i}", tag="proj")
                for hp in range(2):
                    o12sb = rp.tile([128, 128], F32, name=f"o12sb{i}{hp}", tag="o12sb")
                    nc.scalar.activation(o12sb[:], oT[hp][:], AF.Copy)
                    osb = rp.tile([128, 128], F32, name=f"osb{i}{hp}", tag="osb")
                    for e in range(2):
                        nc.vector.tensor_add(osb[64 * e:64 * (e + 1), :],
                                             o3T[hp][64 * e:64 * (e + 1), e:256:2],
                                             o12sb[64 * e:64 * (e + 1), :])
                    nc.tensor.transpose(ops[:, 128 * hp:128 * (hp + 1)], osb[:], ident)

                # residual add with per-head softmax normalization folded in
                seqs1 = acp.tile([128, 256], F32, name=f"seqs1_{i}", tag=f"seqs1_{i}")
                for h in range(4):
                    sl = slice(64 * h, 64 * (h + 1))
                    nc.vector.scalar_tensor_tensor(
                        out=seqs1[:, sl], in0=ops[:, sl], scalar=rr[h][:],
                        in1=seqs_cur[:, sl], op0=OP.mult, op1=OP.add)

                # ---- FFN ----
                rs2 = rmsnorm_rs(i, seqs1, "f")
                h2 = rp.tile([128, 256], F32, name=f"h2_{i}", tag="x")
                nc.vector.tensor_scalar_mul(h2[:], seqs1[:], rs2[:])
                h2T = transpose_pair(i, h2, "h", BF)
                f1 = []
                for t in range(2):
                    pp = psp.tile([128, 128], F32, name=f"f1ps{i}{t}", tag="proj")
                    for a in range(2):
                        nc.tensor.matmul(pp[:], w1(i, a)[:, 128 * t:128 * (t + 1)],
                                         h2T[a][:], start=(a == 0), stop=(a == 1))
                    ft = acp.tile([128, 128], BF, name=f"f1_{i}{t}", tag=f"f1_{i}{t}")
                    nc.scalar.activation(ft[:], pp[:], AF.Relu, bias=b1(i, t))
                    f1.append(ft)
                f2 = []
                for t in range(2):
                    pp = psp.tile([128, 128], F32, name=f"f2ps{i}{t}", tag="proj")
                    for a in range(2):
                        nc.tensor.matmul(pp[:], w2(i, a)[:, 128 * t:128 * (t + 1)],
                                         f1[a][:], start=(a == 0), stop=(a == 1))
                    ft = acp.tile([128, 128], F32, name=f"f2_{i}{t}", tag=f"f2_{i}{t}")
                    nc.scalar.activation(ft[:], pp[:], AF.Identity, bias=b2(i, t))
                    f2.append(ft)
                ftok = psp.tile([128, 256], F32, name=f"ftok{i}", tag="proj")
                for t in range(2):
                    nc.tensor.transpose(ftok[:, 128 * t:128 * (t + 1)], f2[t][:], ident)
                seqs2 = acp.tile([128, 256], F32, name=f"seqs2_{i}", tag=f"seqs2_{i}")
                nc.vector.tensor_add(seqs2[:], seqs1[:], ftok[:])
                nc.vector.tensor_scalar_mul(seqs2[:], seqs2[:], npad)
                seqs_cur = seqs2

                if i == 0:
                    # exchange updated halves within the core pair; the PE
                    # transposes that consume hf are deferred into block 2 so
                    # block-2 w3 matmuls run during the collective
                    gin = dpool.tile([128, 256], F32, name="gin", tag="gin")
                    gout = dpool.tile([2, 128, 256], F32, name="gout", tag="gout")
                    nc.sync.dma_start(out=gin[:], in_=seqs2[:])
                    nc.gpsimd.collective_compute(
                        "AllGather", OP.bypass, replica_groups=AG_GROUPS,
                        ins=[gin.opt()], outs=[gout.opt()])
                    hf = []
                    for half in range(2):
                        t = acp.tile([128, 256], F32, name=f"hf{half}", tag=f"hf{half}")
                        nc.sync.dma_start(out=t[:], in_=gout[half])
                        hf.append(t)

            # ---- final rmsnorm ----
            rs3 = rmsnorm_rs(2, seqs_cur, "o")
            of = acp.tile([128, 256], F32, name="of", tag="of")
            nc.vector.tensor_scalar_mul(of[:], seqs_cur[:], rs3[:])
            nc.vector.tensor_mul(of[:], of[:], lnl)
            nc.sync.dma_start(out=d_out, in_=of[:])

    nc.compile()
    return nc


_CACHE = {}


def _get_nc():
    if "nc" not in _CACHE:
        _CACHE["nc"] = _build()
    return _CACHE["nc"]


def _run(inputs, trace=False, tmpdir=None):
    in_maps = [_prep_core(inputs, c) for c in range(NC)]
    nc = _get_nc()
    res = run_bass_kernel_spmd(nc, in_maps, list(range(NC)), trace=trace, tmpdir=tmpdir)
    out = np.zeros((B, L, H), np.float32)
    for c in range(NC):
        out[c // 2, (c % 2) * 128:(c % 2) * 128 + 128, :] = np.asarray(
            res.results[c]["out"], np.float32)
    return out, res


def kernel(**inputs):
    out, _ = _run(inputs, trace=False)
    return out
